# revision 2
# baseline (speedup 1.0000x reference)
"""GatedCrossAttention Trainium2 kernel.

Sharding: 8 cores = 2 batches x 4 head-groups (8 heads of 32 each).
Each core computes q/k/v projections for its 8 heads (640 of 2560 cols),
attention + softmax, its slice of attn_weights, and a partial o_proj
(row-parallel); host sums the 4 partials per batch and concatenates
attn_weights. gate=tanh(gate_param) is folded into Wo on host.

All matmuls run in bf16 (fp32 PSUM accumulation); outputs are fp32.
"""
import math
import numpy as np
import ml_dtypes

import concourse.bass as bass
import concourse.tile as tile
from concourse import bacc, mybir
from concourse import bass_utils

# problem constants (hardcoded per contract)
B, S, L, D = 2, 2048, 512, 2560
H_TOT, DH = 32, 80
N_CORES = 8
H = 8                 # heads per core
GCOL = H * DH         # 640 columns per head-group
KO = D // 128         # 20 contraction chunks for projections
OKO = GCOL // 128     # 5 contraction chunks for o_proj
SC = 4                # S chunks of 512
ST = 4                # 128-row tiles per S chunk
BF16 = mybir.dt.bfloat16
F32 = mybir.dt.float32
SOFTMAX_SCALE = 1.0 / math.sqrt(DH)

_cache = {}


def _repack_segments():
    """(src_dh, src_h, dst_ki, dst_ko, length) segments mapping rows
    r=80*h+dh of [80,8,*] onto rows r=128*ko+ki of [128,5,*]."""
    segs = []
    r = 0
    while r < GCOL:
        h, dh = divmod(r, DH)
        ko, ki = divmod(r, 128)
        n = min(DH - dh, 128 - ki)
        segs.append((dh, h, ki, ko, n))
        r += n
    return segs


def _build():
    nc = bacc.Bacc("TRN2", target_bir_lowering=False, debug=False,
                   enable_asserts=False, num_devices=N_CORES)
    ht = nc.dram_tensor("ht", [D, S], BF16, kind="ExternalInput").ap()
    lt = nc.dram_tensor("lt", [D, L], BF16, kind="ExternalInput").ap()
    wqt = nc.dram_tensor("wqt", [D, GCOL], BF16, kind="ExternalInput").ap()
    wkt = nc.dram_tensor("wkt", [D, GCOL], BF16, kind="ExternalInput").ap()
    wvt = nc.dram_tensor("wvt", [D, GCOL], BF16, kind="ExternalInput").ap()
    wot = nc.dram_tensor("wot", [GCOL, D], BF16, kind="ExternalInput").ap()
    out_part = nc.dram_tensor("out_part", [S, D], F32, kind="ExternalOutput").ap()
    attn_w = nc.dram_tensor("attn_w", [H, S, L], F32, kind="ExternalOutput").ap()

    kchunks = lambda ap: ap.rearrange("(ko ki) m -> ki ko m", ki=128)

    with tile.TileContext(nc) as tc:
        with tc.tile_pool(name="wts", bufs=1) as wpool, \
             tc.tile_pool(name="qkv", bufs=1) as qkv:
            wo_sb = wpool.tile([128, OKO, D], BF16, tag="wo")
            nc.sync.dma_start(wo_sb[:], kchunks(wot))
            qt_sb = qkv.tile([DH, H, S], BF16, tag="qt")
            kt_sb = qkv.tile([DH, H, L], BF16, tag="kt")
            v_sb = qkv.tile([128, L // 128, GCOL], BF16, tag="v")

            # ---------------- pass 1: projections ----------------
            with tc.tile_pool(name="p1w", bufs=1) as p1w, \
                 tc.tile_pool(name="stream", bufs=2) as stream, \
                 tc.tile_pool(name="p1ps", bufs=4, space="PSUM") as p1ps:
                wq_sb = p1w.tile([128, KO, GCOL], BF16, tag="wq")
                wk_sb = p1w.tile([128, KO, GCOL], BF16, tag="wk")
                wv_sb = p1w.tile([128, KO, GCOL], BF16, tag="wv")
                nc.sync.dma_start(wk_sb[:], kchunks(wkt))
                nc.sync.dma_start(wv_sb[:], kchunks(wvt))
                nc.sync.dma_start(wq_sb[:], kchunks(wqt))
                lt_sb = stream.tile([128, KO, 512], BF16, tag="chunk")
                nc.sync.dma_start(lt_sb[:], kchunks(lt))

                # K projection: kt[dh, h, l] (per-head partitions)
                for h in range(H):
                    ps = p1ps.tile([128, 512], F32, tag="ps", name="ps")[:DH]
                    for k in range(KO):
                        nc.tensor.matmul(ps, lhsT=wk_sb[:, k, h * DH:(h + 1) * DH],
                                         rhs=lt_sb[:, k, :],
                                         start=(k == 0), stop=(k == KO - 1))
                    nc.vector.tensor_copy(kt_sb[:, h, :], ps)
                # V: normal orientation [l, (h dh)] via lhsT = lt chunks
                for lc in range(L // 128):
                    for n0, nsz in ((0, 512), (512, 128)):
                        ps = p1ps.tile([128, 512], F32, tag="ps", name="ps")[:, :nsz]
                        for k in range(KO):
                            nc.tensor.matmul(ps, lhsT=lt_sb[:, k, lc * 128:(lc + 1) * 128],
                                             rhs=wv_sb[:, k, n0:n0 + nsz],
                                             start=(k == 0), stop=(k == KO - 1))
                        nc.vector.tensor_copy(v_sb[:, lc, n0:n0 + nsz], ps)
                # Q projection per S chunk
                for sc in range(SC):
                    ht_sb = stream.tile([128, KO, 512], BF16, tag="chunk")
                    nc.sync.dma_start(ht_sb[:], kchunks(ht)[:, :, sc * 512:(sc + 1) * 512])
                    for h in range(H):
                        ps = p1ps.tile([128, 512], F32, tag="ps", name="ps")[:DH]
                        for k in range(KO):
                            nc.tensor.matmul(ps, lhsT=wq_sb[:, k, h * DH:(h + 1) * DH],
                                             rhs=ht_sb[:, k, :],
                                             start=(k == 0), stop=(k == KO - 1))
                        nc.vector.tensor_copy(qt_sb[:, h, sc * 512:(sc + 1) * 512], ps)

            # ---------------- pass 2: attention + o_proj ----------------
            with tc.tile_pool(name="att", bufs=3) as att, \
                 tc.tile_pool(name="attT", bufs=2) as attTp, \
                 tc.tile_pool(name="aot", bufs=2) as aotp, \
                 tc.tile_pool(name="rep", bufs=2) as repp, \
                 tc.tile_pool(name="ost", bufs=4) as ostp, \
                 tc.tile_pool(name="sm", bufs=6) as smp, \
                 tc.tile_pool(name="scps", bufs=2, space="PSUM") as scps, \
                 tc.tile_pool(name="pvps", bufs=2, space="PSUM") as pvps, \
                 tc.tile_pool(name="ops", bufs=3, space="PSUM") as ops:
                for sc in range(SC):
                    aot_sb = aotp.tile([DH, H, 512], BF16, tag="aot")
                    for h in range(H):
                        attnT = attTp.tile([128, L // 128, 512], BF16, tag="attnT")
                        for st in range(ST):
                            s0 = sc * 512 + st * 128
                            scp = scps.tile([128, 512], F32, tag="sc")
                            nc.tensor.matmul(scp, lhsT=qt_sb[:, h, s0:s0 + 128],
                                             rhs=kt_sb[:, h, :], start=True, stop=True)
                            expt = att.tile([128, 512], F32, tag="exp")
                            sums = smp.tile([128, 1], F32, tag="sums")
                            nc.scalar.activation(expt, scp,
                                                 mybir.ActivationFunctionType.Exp,
                                                 scale=SOFTMAX_SCALE, accum_out=sums)
                            rec = smp.tile([128, 1], F32, tag="rec")
                            nc.vector.reciprocal(rec, sums)
                            attnf = att.tile([128, 512], F32, tag="attnf")
                            nc.scalar.activation(attnf, expt,
                                                 mybir.ActivationFunctionType.Copy,
                                                 scale=rec)
                            nc.sync.dma_start(attn_w[h, s0:s0 + 128, :], attnf)
                            attnb = att.tile([128, 512], BF16, tag="attnb")
                            nc.vector.tensor_scalar_mul(attnb, expt, rec)
                            for lc in range(L // 128):
                                nc.sync.dma_start_transpose(
                                    attnT[:, lc, st * 128:(st + 1) * 128],
                                    attnb[:, lc * 128:(lc + 1) * 128])
                        pvp = pvps.tile([128, 512], F32, tag="pv", name="pvp")[:DH]
                        for lc in range(L // 128):
                            nc.tensor.matmul(pvp, lhsT=v_sb[:, lc, h * DH:(h + 1) * DH],
                                             rhs=attnT[:, lc, :],
                                             start=(lc == 0), stop=(lc == L // 128 - 1))
                        nc.vector.tensor_copy(aot_sb[:, h, :], pvp)
                    rep_sb = repp.tile([128, OKO, 512], BF16, tag="rep")
                    for (dh, h, ki, ko, n) in _repack_segments():
                        nc.sync.dma_start(rep_sb[ki:ki + n, ko, :],
                                          aot_sb[dh:dh + n, h, :])
                    for st in range(ST):
                        s0 = sc * 512 + st * 128
                        for nb in range(D // 512):
                            op = ops.tile([128, 512], F32, tag="op")
                            for ko in range(OKO):
                                nc.tensor.matmul(op, lhsT=rep_sb[:, ko, st * 128:(st + 1) * 128],
                                                 rhs=wo_sb[:, ko, nb * 512:(nb + 1) * 512],
                                                 start=(ko == 0), stop=(ko == OKO - 1))
                            ost = ostp.tile([128, 512], F32, tag="ost")
                            nc.scalar.copy(ost, op)
                            nc.sync.dma_start(
                                out_part[s0:s0 + 128, nb * 512:(nb + 1) * 512], ost)
    nc.compile()
    return nc


def kernel(hidden_states, live_state, Wq, Wk, Wv, Wo, gate_param):
    hidden_states = np.asarray(hidden_states, dtype=np.float32)
    live_state = np.asarray(live_state, dtype=np.float32)
    Wq = np.asarray(Wq, dtype=np.float32)
    Wk = np.asarray(Wk, dtype=np.float32)
    Wv = np.asarray(Wv, dtype=np.float32)
    Wo = np.asarray(Wo, dtype=np.float32)
    gate = float(np.tanh(np.asarray(gate_param, dtype=np.float32)))

    bf = ml_dtypes.bfloat16
    hts = [np.ascontiguousarray(hidden_states[b].T).astype(bf) for b in range(B)]
    lts = [np.ascontiguousarray(live_state[b].T).astype(bf) for b in range(B)]
    wqts, wkts, wvts, wots = [], [], [], []
    for g in range(4):
        rows = slice(g * GCOL, (g + 1) * GCOL)
        wqts.append(np.ascontiguousarray(Wq[rows, :].T).astype(bf))
        wkts.append(np.ascontiguousarray(Wk[rows, :].T).astype(bf))
        wvts.append(np.ascontiguousarray(Wv[rows, :].T).astype(bf))
        wots.append(np.ascontiguousarray(Wo[:, rows].T * gate).astype(bf))

    if "nc" not in _cache:
        _cache["nc"] = _build()
    nc = _cache["nc"]

    in_maps = []
    for c in range(N_CORES):
        b, g = divmod(c, 4)
        in_maps.append({"ht": hts[b], "lt": lts[b], "wqt": wqts[g],
                        "wkt": wkts[g], "wvt": wvts[g], "wot": wots[g]})
    res = bass_utils.run_bass_kernel_spmd(nc, in_maps, core_ids=list(range(N_CORES)))

    output = np.zeros((B, S, D), dtype=np.float32)
    attn = np.zeros((B, H_TOT, S, L), dtype=np.float32)
    for c in range(N_CORES):
        b, g = divmod(c, 4)
        output[b] += res.results[c]["out_part"]
        attn[b, g * H:(g + 1) * H] = res.results[c]["attn_w"]
    return output, attn
